# revision 3
# baseline (speedup 1.0000x reference)
"""Trainium2 Bass kernel for nn_LocalEncoderSDESepPara2 (SDE+GRU scan).

Full-input contract: kernel(**inputs) takes the unsharded numpy inputs and
returns (drift_loc, all_drift_loc, all_diff_z) exactly like the reference.

Sharding: data-parallel over B (16 batches -> 2 per core on 8 cores).  The
(C,C) weights are replicated; the 21-step scan runs locally per core.

Device math (per core, per step, state h: [C=128 partitions, 2048 cols]):
  drift  = tanh(Wf@h + bf)                (ACT, PSUM->SBUF bf16)
  diff   = sigmoid(Wg@h + bg)             (ACT, bf16)
  dz     = diff * (noise*sqrt|dt|)        (DVE bf16 2x; also an output)
  S      = (dt*I)@drift + I@dz            (PE identity matmuls, accumulated)
  h_sde  = h + S                          (DVE fp32)
  z      = sigmoid(Wz@x + Uz@h_sde + bz)  (PE accum + ACT, fp32)
  r      = sigmoid(Wr@x + Ur@h_sde + br)  (fp32)
  rU     = r * (Uh@h_sde)                 (DVE, out bf16)
  n      = tanh(Wh@x + I@rU + bh)         (PE accum + ACT, fp32)
  d      = h_sde - n                      (GPSIMD fp32)
  h_new  = n + z*d                        (DVE fp32)
Matmuls run in bf16 (inputs rounded), recurrent state stays fp32.
"""

import numpy as np
import ml_dtypes
from contextlib import ExitStack

B, C, TP, PN, STEPS = 16, 128, 11, 1024, 21
N_CORES = 8
BL = B // N_CORES          # batches per core = halves per core
F = BL * PN                # 2048 free columns per core
bf16 = ml_dtypes.bfloat16

# stationary-weight slots in the packed weight tensor
WF, WG, WZ, UZ, WR, UR, WH, UH, I_DT0, I_DTN, I_ID = range(11)
NW = 11

_CACHE = {}


def _dts():
    ts = np.linspace(TP - 1.0, 0.0, STEPS).astype(np.float32)
    return np.concatenate([np.full((1,), 0.01, np.float32), np.diff(ts)])


def _build_nc():
    import concourse.bacc as bacc
    import concourse.tile as tile
    from concourse import mybir

    dt = mybir.dt
    AF = mybir.ActivationFunctionType

    nc = bacc.Bacc("TRN2", target_bir_lowering=False, debug=False,
                   num_devices=N_CORES)
    aa = nc.dram_tensor("aa", (STEPS, BL, C, PN), dt.bfloat16,
                        kind="ExternalInput").ap()
    nts = nc.dram_tensor("nts", (STEPS, BL, C, PN), dt.bfloat16,
                         kind="ExternalInput").ap()
    h0f_d = nc.dram_tensor("h0f", (C, PN), dt.float32,
                           kind="ExternalInput").ap()
    h0b_d = nc.dram_tensor("h0b", (C, PN), dt.bfloat16,
                           kind="ExternalInput").ap()
    wt_d = nc.dram_tensor("wt", (NW, C, C), dt.bfloat16,
                          kind="ExternalInput").ap()
    bias_d = nc.dram_tensor("bias", (C, 5), dt.float32,
                            kind="ExternalInput").ap()
    drift_d = nc.dram_tensor("drift", (STEPS, BL, C, PN), dt.float32,
                             kind="ExternalOutput").ap()
    dz_d = nc.dram_tensor("dz", (STEPS, BL, C, PN), dt.bfloat16,
                          kind="ExternalOutput").ap()

    with tile.TileContext(nc) as tc, ExitStack() as ctx:
        const = ctx.enter_context(tc.tile_pool(name="const", bufs=1))
        wt = const.tile([C, NW * C], dt.bfloat16)
        nc.sync.dma_start(
            wt[:].rearrange("c (i o) -> c i o", i=NW),
            wt_d.rearrange("i c o -> c i o"),
        )
        bias = const.tile([C, 5], dt.float32)
        nc.sync.dma_start(bias[:], bias_d[:])
        h0f = const.tile([C, F], dt.float32)
        for hh in range(BL):
            nc.sync.dma_start(h0f[:, hh * PN:(hh + 1) * PN], h0f_d[:])
        h0b = const.tile([C, F], dt.bfloat16)
        for hh in range(BL):
            nc.sync.dma_start(h0b[:, hh * PN:(hh + 1) * PN], h0b_d[:])

        iop = ctx.enter_context(tc.tile_pool(name="io", bufs=2))
        gp = ctx.enter_context(tc.tile_pool(name="gates", bufs=2))
        sp = ctx.enter_context(tc.tile_pool(name="state", bufs=2))
        hp = ctx.enter_context(tc.tile_pool(name="hstate", bufs=2))
        psp = ctx.enter_context(tc.tile_pool(name="psum", bufs=4, space="PSUM"))

        def W(i):
            return wt[:, i * C:(i + 1) * C]

        def mm2(ps, iw, rhs, start, stop):
            # ps: (C, PN) psum tile; rhs: (C, F) tile sliced per half `rhs_off`
            for s in range(2):
                nc.tensor.matmul(ps[:, s * 512:(s + 1) * 512], W(iw),
                                 rhs[:, s * 512:(s + 1) * 512],
                                 start=start, stop=stop)

        h_fp = h0f
        h_bf = h0b

        for t in range(STEPS):
            idt = I_DT0 if t == 0 else I_DTN
            xt = iop.tile([C, F], dt.bfloat16, tag="xt")
            nc.sync.dma_start(
                xt[:].rearrange("c (b p) -> c b p", b=BL),
                aa[t].rearrange("b c p -> c b p"))
            nt = iop.tile([C, F], dt.bfloat16, tag="nt")
            nc.sync.dma_start(
                nt[:].rearrange("c (b p) -> c b p", b=BL),
                nts[t].rearrange("b c p -> c b p"))

            dzb = gp.tile([C, F], dt.bfloat16, tag="dz")
            hsde = sp.tile([C, F], dt.float32, tag="hsde")
            hsb = gp.tile([C, F], dt.bfloat16, tag="hsb")
            nf = sp.tile([C, F], dt.float32, tag="n")
            df = sp.tile([C, F], dt.float32, tag="d")
            hnew = hp.tile([C, F], dt.float32, tag="hnew")
            hnb = hp.tile([C, F], dt.bfloat16, tag="hnb")

            for hh in range(BL):
                hl = slice(hh * PN, (hh + 1) * PN)

                psF = psp.tile([C, PN], dt.float32, tag="ps")
                mm2(psF, WF, h_bf[:, hl], True, True)
                psG = psp.tile([C, PN], dt.float32, tag="ps")
                mm2(psG, WG, h_bf[:, hl], True, True)
                drift = gp.tile([C, PN], dt.bfloat16, tag="drift")
                nc.scalar.activation(drift[:], psF[:], AF.Tanh,
                                     bias=bias[:, 0:1])
                diff = gp.tile([C, PN], dt.bfloat16, tag="diff")
                nc.scalar.activation(diff[:], psG[:], AF.Sigmoid,
                                     bias=bias[:, 1:2])
                nc.vector.tensor_mul(dzb[:, hl], diff[:], nt[:, hl])

                psS = psp.tile([C, PN], dt.float32, tag="ps")
                mm2(psS, idt, drift[:], True, False)
                mm2(psS, I_ID, dzb[:, hl], False, True)
                nc.vector.tensor_add(hsde[:, hl], h_fp[:, hl], psS[:])
                nc.vector.tensor_copy(hsb[:, hl], hsde[:, hl])

                psZ = psp.tile([C, PN], dt.float32, tag="ps")
                mm2(psZ, WZ, xt[:, hl], True, False)
                mm2(psZ, UZ, hsb[:, hl], False, True)
                zb = gp.tile([C, PN], dt.float32, tag="z")
                nc.scalar.activation(zb[:], psZ[:], AF.Sigmoid,
                                     bias=bias[:, 2:3])

                psR = psp.tile([C, PN], dt.float32, tag="ps")
                mm2(psR, WR, xt[:, hl], True, False)
                mm2(psR, UR, hsb[:, hl], False, True)
                rb = gp.tile([C, PN], dt.float32, tag="r")
                nc.scalar.activation(rb[:], psR[:], AF.Sigmoid,
                                     bias=bias[:, 3:4])

                psHH = psp.tile([C, PN], dt.float32, tag="ps")
                mm2(psHH, UH, hsb[:, hl], True, True)
                ru = gp.tile([C, PN], dt.bfloat16, tag="ru")
                nc.vector.tensor_mul(ru[:], rb[:], psHH[:])

                psHX = psp.tile([C, PN], dt.float32, tag="ps")
                mm2(psHX, WH, xt[:, hl], True, False)
                mm2(psHX, I_ID, ru[:], False, True)
                nc.scalar.activation(nf[:, hl], psHX[:], AF.Tanh,
                                     bias=bias[:, 4:5])

                nc.gpsimd.tensor_sub(df[:, hl], hsde[:, hl], nf[:, hl])
                zd = sp.tile([C, PN], dt.float32, tag="zd")
                nc.vector.tensor_mul(zd[:], zb[:], df[:, hl])
                nc.vector.tensor_add(hnew[:, hl], zd[:], nf[:, hl])
                nc.gpsimd.tensor_copy(hnb[:, hl], hnew[:, hl])

            nc.sync.dma_start(
                dz_d[t].rearrange("b c p -> c b p"),
                dzb[:].rearrange("c (b p) -> c b p", b=BL))
            nc.sync.dma_start(
                drift_d[t].rearrange("b c p -> c b p"),
                hnew[:].rearrange("c (b p) -> c b p", b=BL))
            h_fp = hnew
            h_bf = hnb

    nc.compile()
    return nc


def _get_nc():
    if "nc" not in _CACHE:
        _CACHE["nc"] = _build_nc()
    return _CACHE["nc"]


def _host_prep(track_loc, aa_out, noise, hidden, Wf, bf, Wg, bg,
               Wz, Uz, bz, Wr, Ur, br, Wh, Uh, bh):
    dts = _dts()
    sq = np.sqrt(np.abs(dts)).astype(np.float32)

    eye = np.eye(C, dtype=np.float32)
    wt = np.stack([
        np.asarray(Wf).T, np.asarray(Wg).T, np.asarray(Wz).T,
        np.asarray(Uz).T, np.asarray(Wr).T, np.asarray(Ur).T,
        np.asarray(Wh).T, np.asarray(Uh).T,
        eye * dts[0], eye * dts[1], eye,
    ]).astype(bf16)
    bias = np.stack([np.asarray(b, np.float32) for b in (bf, bg, bz, br, bh)],
                    axis=1)
    h0f = np.ascontiguousarray(
        np.broadcast_to(np.asarray(hidden, np.float32)[:, None], (C, PN)))
    h0b = h0f.astype(bf16)

    aa_bf = np.asarray(aa_out, np.float32).astype(bf16)
    nts_bf = (np.asarray(noise, np.float32)
              * sq[:, None, None, None]).astype(bf16)

    in_maps = []
    for k in range(N_CORES):
        sl = slice(BL * k, BL * (k + 1))
        in_maps.append({
            "aa": np.ascontiguousarray(aa_bf[:, sl]),
            "nts": np.ascontiguousarray(nts_bf[:, sl]),
            "h0f": h0f, "h0b": h0b, "wt": wt, "bias": bias,
        })
    return in_maps


def _assemble(results):
    hs = np.concatenate([r["drift"] for r in results], axis=1)   # (S,B,C,PN) f32
    dzs = np.concatenate([r["dz"].astype(np.float32) for r in results], axis=1)
    all_drift = np.ascontiguousarray(hs.transpose(1, 0, 2, 3))
    all_dz = np.ascontiguousarray(dzs.transpose(1, 0, 2, 3))
    stride = (STEPS - 1) // (TP - 1)
    drift_loc = np.ascontiguousarray(all_drift[:, ::stride])
    return drift_loc, all_drift, all_dz


def kernel(**inputs):
    from concourse import bass_utils
    nc = _get_nc()
    in_maps = _host_prep(**inputs)
    res = bass_utils.run_bass_kernel_spmd(nc, in_maps,
                                          core_ids=list(range(N_CORES)))
    return _assemble(res.results)


# revision 5
# speedup vs baseline: 1.3728x; 1.3728x over previous
"""Trainium2 Bass kernel for nn_LocalEncoderSDESepPara2 (SDE+GRU scan).

Full-input contract: kernel(**inputs) takes the unsharded numpy inputs and
returns (drift_loc, all_drift_loc, all_diff_z) exactly like the reference.

Sharding: data-parallel over B (16 batches -> 2 per core on 8 cores).  The
(C,C) weights are replicated; the 21-step scan runs locally per core.

Device math (per core, per step, state h: [C=128 partitions, 2048 cols]):
  drift  = tanh(Wf@h + bf)                (ACT, PSUM->SBUF bf16)
  diff   = sigmoid(Wg@h + bg)             (ACT, bf16)
  dz     = diff * (noise*sqrt|dt|)        (DVE bf16 2x; also an output)
  S      = (dt*I)@drift + I@dz            (PE identity matmuls, accumulated)
  h_sde  = h + S                          (DVE fp32)
  z      = sigmoid(Wz@x + Uz@h_sde + bz)  (PE accum + ACT, fp32)
  r      = sigmoid(Wr@x + Ur@h_sde + br)  (fp32)
  rU     = r * (Uh@h_sde)                 (DVE, out bf16)
  n      = tanh(Wh@x + I@rU + bh)         (PE accum + ACT, fp32)
  d      = h_sde - n                      (GPSIMD fp32)
  h_new  = n + z*d                        (DVE fp32)
Matmuls run in bf16 (inputs rounded), recurrent state stays fp32.
"""

import numpy as np
import ml_dtypes
from contextlib import ExitStack

B, C, TP, PN, STEPS = 16, 128, 11, 1024, 21
N_CORES = 8
BL = B // N_CORES          # batches per core = halves per core
F = BL * PN                # 2048 free columns per core
bf16 = ml_dtypes.bfloat16

# stationary-weight slots in the packed weight tensor
WF, WG, WZ, UZ, WR, UR, WH, UH, I_DT0, I_DTN, I_ID = range(11)
NW = 11

_CACHE = {}


def _dts():
    ts = np.linspace(TP - 1.0, 0.0, STEPS).astype(np.float32)
    return np.concatenate([np.full((1,), 0.01, np.float32), np.diff(ts)])


def _build_nc():
    import concourse.bacc as bacc
    import concourse.tile as tile
    from concourse import mybir

    dt = mybir.dt
    AF = mybir.ActivationFunctionType

    nc = bacc.Bacc("TRN2", target_bir_lowering=False, debug=False,
                   num_devices=N_CORES)
    aa = nc.dram_tensor("aa", (STEPS, BL, C, PN), dt.bfloat16,
                        kind="ExternalInput").ap()
    nts = nc.dram_tensor("nts", (STEPS, BL, C, PN), dt.bfloat16,
                         kind="ExternalInput").ap()
    h0f_d = nc.dram_tensor("h0f", (C, PN), dt.float32,
                           kind="ExternalInput").ap()
    h0b_d = nc.dram_tensor("h0b", (C, PN), dt.bfloat16,
                           kind="ExternalInput").ap()
    wt_d = nc.dram_tensor("wt", (NW, C, C), dt.bfloat16,
                          kind="ExternalInput").ap()
    bias_d = nc.dram_tensor("bias", (C, 5), dt.float32,
                            kind="ExternalInput").ap()
    drift_d = nc.dram_tensor("drift", (STEPS, BL, C, PN), dt.float32,
                             kind="ExternalOutput").ap()
    dz_d = nc.dram_tensor("dz", (STEPS, BL, C, PN), dt.bfloat16,
                          kind="ExternalOutput").ap()

    with tile.TileContext(nc) as tc, ExitStack() as ctx:
        const = ctx.enter_context(tc.tile_pool(name="const", bufs=1))
        wt = const.tile([C, NW * C], dt.bfloat16)
        nc.sync.dma_start(
            wt[:].rearrange("c (i o) -> c i o", i=NW),
            wt_d.rearrange("i c o -> c i o"),
        )
        bias = const.tile([C, 5], dt.float32)
        nc.sync.dma_start(bias[:], bias_d[:])
        h0f = const.tile([C, F], dt.float32)
        for hh in range(BL):
            nc.sync.dma_start(h0f[:, hh * PN:(hh + 1) * PN], h0f_d[:])
        h0b = const.tile([C, F], dt.bfloat16)
        for hh in range(BL):
            nc.sync.dma_start(h0b[:, hh * PN:(hh + 1) * PN], h0b_d[:])

        iop = ctx.enter_context(tc.tile_pool(name="io", bufs=2))
        gp = ctx.enter_context(tc.tile_pool(name="gates", bufs=2))
        sp = ctx.enter_context(tc.tile_pool(name="state", bufs=2))
        hp = ctx.enter_context(tc.tile_pool(name="hstate", bufs=2))
        psA = ctx.enter_context(tc.tile_pool(name="psumA", bufs=2, space="PSUM"))
        psB = ctx.enter_context(tc.tile_pool(name="psumB", bufs=2, space="PSUM"))

        def W(i):
            return wt[:, i * C:(i + 1) * C]

        def mm2(ps, iw, rhs, start, stop):
            # ps: (C, PN) psum tile; rhs: (C, F) tile sliced per half `rhs_off`
            for s in range(2):
                nc.tensor.matmul(ps[:, s * 512:(s + 1) * 512], W(iw),
                                 rhs[:, s * 512:(s + 1) * 512],
                                 start=start, stop=stop)

        h_fp = h0f
        h_bf = h0b

        for t in range(STEPS):
            idt = I_DT0 if t == 0 else I_DTN
            xt = iop.tile([C, F], dt.bfloat16, tag="xt")
            nc.sync.dma_start(
                xt[:].rearrange("c (b p) -> c b p", b=BL),
                aa[t].rearrange("b c p -> c b p"))
            nt = iop.tile([C, F], dt.bfloat16, tag="nt")
            nc.sync.dma_start(
                nt[:].rearrange("c (b p) -> c b p", b=BL),
                nts[t].rearrange("b c p -> c b p"))

            dzb = gp.tile([C, F], dt.bfloat16, tag="dz")
            hsde = sp.tile([C, F], dt.float32, tag="hsde")
            hsb = gp.tile([C, F], dt.bfloat16, tag="hsb")
            nf = sp.tile([C, F], dt.float32, tag="n")
            df = sp.tile([C, F], dt.float32, tag="d")
            hnew = hp.tile([C, F], dt.float32, tag="hnew")
            hnb = hp.tile([C, F], dt.bfloat16, tag="hnb")

            for hh in range(BL):
                hl = slice(hh * PN, (hh + 1) * PN)
                psp = psA if hh == 0 else psB
                pst = "psA" if hh == 0 else "psB"

                psG = psp.tile([C, PN], dt.float32, tag=pst)
                mm2(psG, WG, h_bf[:, hl], True, True)
                psF = psp.tile([C, PN], dt.float32, tag=pst)
                mm2(psF, WF, h_bf[:, hl], True, True)
                diff = gp.tile([C, PN], dt.bfloat16, tag="diff")
                nc.scalar.activation(diff[:], psG[:], AF.Sigmoid,
                                     bias=bias[:, 1:2])
                drift = gp.tile([C, PN], dt.bfloat16, tag="drift")
                nc.scalar.activation(drift[:], psF[:], AF.Tanh,
                                     bias=bias[:, 0:1])
                nc.vector.tensor_mul(dzb[:, hl], diff[:], nt[:, hl])

                psS = psp.tile([C, PN], dt.float32, tag=pst)
                mm2(psS, idt, drift[:], True, False)
                mm2(psS, I_ID, dzb[:, hl], False, True)
                nc.vector.tensor_add(hsb[:, hl], h_fp[:, hl], psS[:])
                nc.vector.tensor_add(hsde[:, hl], h_fp[:, hl], psS[:])

                psR = psp.tile([C, PN], dt.float32, tag=pst)
                mm2(psR, WR, xt[:, hl], True, False)
                mm2(psR, UR, hsb[:, hl], False, True)
                rb = gp.tile([C, PN], dt.float32, tag="r")
                nc.scalar.activation(rb[:], psR[:], AF.Sigmoid,
                                     bias=bias[:, 3:4])

                psHH = psp.tile([C, PN], dt.float32, tag=pst)
                mm2(psHH, UH, hsb[:, hl], True, True)
                ru = gp.tile([C, PN], dt.bfloat16, tag="ru")
                nc.vector.tensor_mul(ru[:], rb[:], psHH[:])

                psHX = psp.tile([C, PN], dt.float32, tag=pst)
                mm2(psHX, WH, xt[:, hl], True, False)
                mm2(psHX, I_ID, ru[:], False, True)
                nc.scalar.activation(nf[:, hl], psHX[:], AF.Tanh,
                                     bias=bias[:, 4:5])

                psZ = psp.tile([C, PN], dt.float32, tag=pst)
                mm2(psZ, WZ, xt[:, hl], True, False)
                mm2(psZ, UZ, hsb[:, hl], False, True)
                zb = gp.tile([C, PN], dt.float32, tag="z")
                nc.scalar.activation(zb[:], psZ[:], AF.Sigmoid,
                                     bias=bias[:, 2:3])

                nc.gpsimd.tensor_sub(df[:, hl], hsde[:, hl], nf[:, hl])
                zd = sp.tile([C, PN], dt.float32, tag="zd")
                nc.vector.tensor_mul(zd[:], zb[:], df[:, hl])
                nc.vector.tensor_add(hnb[:, hl], zd[:], nf[:, hl])
                nc.gpsimd.tensor_add(hnew[:, hl], zd[:], nf[:, hl])

            nc.sync.dma_start(
                dz_d[t].rearrange("b c p -> c b p"),
                dzb[:].rearrange("c (b p) -> c b p", b=BL))
            nc.sync.dma_start(
                drift_d[t].rearrange("b c p -> c b p"),
                hnew[:].rearrange("c (b p) -> c b p", b=BL))
            h_fp = hnew
            h_bf = hnb

    nc.compile()
    return nc


def _get_nc():
    if "nc" not in _CACHE:
        _CACHE["nc"] = _build_nc()
    return _CACHE["nc"]


def _host_prep(track_loc, aa_out, noise, hidden, Wf, bf, Wg, bg,
               Wz, Uz, bz, Wr, Ur, br, Wh, Uh, bh):
    dts = _dts()
    sq = np.sqrt(np.abs(dts)).astype(np.float32)

    eye = np.eye(C, dtype=np.float32)
    wt = np.stack([
        np.asarray(Wf).T, np.asarray(Wg).T, np.asarray(Wz).T,
        np.asarray(Uz).T, np.asarray(Wr).T, np.asarray(Ur).T,
        np.asarray(Wh).T, np.asarray(Uh).T,
        eye * dts[0], eye * dts[1], eye,
    ]).astype(bf16)
    bias = np.stack([np.asarray(b, np.float32) for b in (bf, bg, bz, br, bh)],
                    axis=1)
    h0f = np.ascontiguousarray(
        np.broadcast_to(np.asarray(hidden, np.float32)[:, None], (C, PN)))
    h0b = h0f.astype(bf16)

    aa_bf = np.asarray(aa_out, np.float32).astype(bf16)
    nts_bf = (np.asarray(noise, np.float32)
              * sq[:, None, None, None]).astype(bf16)

    in_maps = []
    for k in range(N_CORES):
        sl = slice(BL * k, BL * (k + 1))
        in_maps.append({
            "aa": np.ascontiguousarray(aa_bf[:, sl]),
            "nts": np.ascontiguousarray(nts_bf[:, sl]),
            "h0f": h0f, "h0b": h0b, "wt": wt, "bias": bias,
        })
    return in_maps


def _assemble(results):
    hs = np.concatenate([r["drift"] for r in results], axis=1)   # (S,B,C,PN) f32
    dzs = np.concatenate([r["dz"].astype(np.float32) for r in results], axis=1)
    all_drift = np.ascontiguousarray(hs.transpose(1, 0, 2, 3))
    all_dz = np.ascontiguousarray(dzs.transpose(1, 0, 2, 3))
    stride = (STEPS - 1) // (TP - 1)
    drift_loc = np.ascontiguousarray(all_drift[:, ::stride])
    return drift_loc, all_drift, all_dz


def kernel(**inputs):
    from concourse import bass_utils
    nc = _get_nc()
    in_maps = _host_prep(**inputs)
    res = bass_utils.run_bass_kernel_spmd(nc, in_maps,
                                          core_ids=list(range(N_CORES)))
    return _assemble(res.results)


# revision 6
# speedup vs baseline: 1.6788x; 1.2229x over previous
"""Trainium2 Bass kernel for nn_LocalEncoderSDESepPara2 (SDE+GRU scan).

Full-input contract: kernel(**inputs) takes the unsharded numpy inputs and
returns (drift_loc, all_drift_loc, all_diff_z) exactly like the reference.

Sharding: data-parallel over B (16 batches -> 2 per core on 8 cores).  The
(C,C) weights are replicated; the 21-step scan runs locally per core.

Device math (per core, per step, state h: [C=128 partitions, 2048 cols]):
  drift  = tanh(Wf@h + bf)                (ACT, PSUM->SBUF bf16)
  diff   = sigmoid(Wg@h + bg)             (ACT, bf16)
  dz     = diff * (noise*sqrt|dt|)        (DVE bf16 2x; also an output)
  S      = (dt*I)@drift + I@dz            (PE identity matmuls, accumulated)
  h_sde  = h + S                          (DVE fp32)
  z      = sigmoid(Wz@x + Uz@h_sde + bz)  (PE accum + ACT, fp32)
  r      = sigmoid(Wr@x + Ur@h_sde + br)  (fp32)
  rU     = r * (Uh@h_sde)                 (DVE, out bf16)
  n      = tanh(Wh@x + I@rU + bh)         (PE accum + ACT, fp32)
  d      = h_sde - n                      (GPSIMD fp32)
  h_new  = n + z*d                        (DVE fp32)
Matmuls run in bf16 (inputs rounded), recurrent state stays fp32.
"""

import numpy as np
import ml_dtypes
from contextlib import ExitStack

B, C, TP, PN, STEPS = 16, 128, 11, 1024, 21
N_CORES = 8
BL = B // N_CORES          # batches per core = halves per core
F = BL * PN                # 2048 free columns per core
bf16 = ml_dtypes.bfloat16

# stationary-weight slots in the packed weight tensor
WF, WG, WZ, UZ, WR, UR, WH, UH, I_DT0, I_DTN, I_ID = range(11)
NW = 11

_CACHE = {}


def _dts():
    ts = np.linspace(TP - 1.0, 0.0, STEPS).astype(np.float32)
    return np.concatenate([np.full((1,), 0.01, np.float32), np.diff(ts)])


def _build_nc():
    import concourse.bacc as bacc
    import concourse.tile as tile
    from concourse import mybir

    dt = mybir.dt
    AF = mybir.ActivationFunctionType

    nc = bacc.Bacc("TRN2", target_bir_lowering=False, debug=False,
                   num_devices=N_CORES)
    aa = nc.dram_tensor("aa", (STEPS, BL, C, PN), dt.bfloat16,
                        kind="ExternalInput").ap()
    nts = nc.dram_tensor("nts", (STEPS, BL, C, PN), dt.bfloat16,
                         kind="ExternalInput").ap()
    h0f_d = nc.dram_tensor("h0f", (C, PN), dt.float32,
                           kind="ExternalInput").ap()
    h0b_d = nc.dram_tensor("h0b", (C, PN), dt.bfloat16,
                           kind="ExternalInput").ap()
    wt_d = nc.dram_tensor("wt", (NW, C, C), dt.bfloat16,
                          kind="ExternalInput").ap()
    bias_d = nc.dram_tensor("bias", (C, 5), dt.float32,
                            kind="ExternalInput").ap()
    drift_d = nc.dram_tensor("drift", (STEPS, BL, C, PN), dt.float32,
                             kind="ExternalOutput").ap()
    dz_d = nc.dram_tensor("dz", (STEPS, BL, C, PN), dt.bfloat16,
                          kind="ExternalOutput").ap()

    with tile.TileContext(nc) as tc, ExitStack() as ctx:
        const = ctx.enter_context(tc.tile_pool(name="const", bufs=1))
        wt = const.tile([C, NW * C], dt.bfloat16)
        nc.sync.dma_start(
            wt[:].rearrange("c (i o) -> c i o", i=NW),
            wt_d.rearrange("i c o -> c i o"),
        )
        bias = const.tile([C, 5], dt.float32)
        nc.sync.dma_start(bias[:], bias_d[:])
        h0f = const.tile([C, F], dt.float32)
        for hh in range(BL):
            nc.sync.dma_start(h0f[:, hh * PN:(hh + 1) * PN], h0f_d[:])
        h0b = const.tile([C, F], dt.bfloat16)
        for hh in range(BL):
            nc.sync.dma_start(h0b[:, hh * PN:(hh + 1) * PN], h0b_d[:])

        iop = ctx.enter_context(tc.tile_pool(name="io", bufs=2))
        gp = ctx.enter_context(tc.tile_pool(name="gates", bufs=2))
        sp = ctx.enter_context(tc.tile_pool(name="state", bufs=2))
        hp = ctx.enter_context(tc.tile_pool(name="hstate", bufs=2))
        psA = ctx.enter_context(tc.tile_pool(name="psumA", bufs=2, space="PSUM"))
        psB = ctx.enter_context(tc.tile_pool(name="psumB", bufs=2, space="PSUM"))

        def W(i):
            return wt[:, i * C:(i + 1) * C]

        def mm2(ps, iw, rhs, start, stop):
            # ps: (C, PN) psum tile; rhs: (C, F) tile sliced per half `rhs_off`
            for s in range(2):
                nc.tensor.matmul(ps[:, s * 512:(s + 1) * 512], W(iw),
                                 rhs[:, s * 512:(s + 1) * 512],
                                 start=start, stop=stop)

        h_fp = h0f
        h_bf = h0b

        for t in range(STEPS):
            idt = I_DT0 if t == 0 else I_DTN
            xt = iop.tile([C, F], dt.bfloat16, tag="xt")
            nc.sync.dma_start(
                xt[:].rearrange("c (b p) -> c b p", b=BL),
                aa[t].rearrange("b c p -> c b p"))
            nt = iop.tile([C, F], dt.bfloat16, tag="nt")
            nc.sync.dma_start(
                nt[:].rearrange("c (b p) -> c b p", b=BL),
                nts[t].rearrange("b c p -> c b p"))

            dzb = gp.tile([C, F], dt.bfloat16, tag="dz")
            hsde = sp.tile([C, F], dt.float32, tag="hsde")
            hsb = gp.tile([C, F], dt.bfloat16, tag="hsb")
            nf = sp.tile([C, F], dt.float32, tag="n")
            hnew = hp.tile([C, F], dt.float32, tag="hnew")
            hnb = hp.tile([C, F], dt.bfloat16, tag="hnb")

            for hh in range(BL):
                hl = slice(hh * PN, (hh + 1) * PN)
                psp = psA if hh == 0 else psB
                pst = "psA" if hh == 0 else "psB"

                psG = psp.tile([C, PN], dt.float32, tag=pst)
                mm2(psG, WG, h_bf[:, hl], True, True)
                psF = psp.tile([C, PN], dt.float32, tag=pst)
                mm2(psF, WF, h_bf[:, hl], True, True)
                diff = gp.tile([C, PN], dt.bfloat16, tag="diff")
                nc.scalar.activation(diff[:], psG[:], AF.Sigmoid,
                                     bias=bias[:, 1:2])
                drift = gp.tile([C, PN], dt.bfloat16, tag="drift")
                nc.scalar.activation(drift[:], psF[:], AF.Tanh,
                                     bias=bias[:, 0:1])
                nc.vector.tensor_mul(dzb[:, hl], diff[:], nt[:, hl])

                psS = psp.tile([C, PN], dt.float32, tag=pst)
                mm2(psS, idt, drift[:], True, False)
                mm2(psS, I_ID, dzb[:, hl], False, True)
                nc.vector.tensor_add(hsb[:, hl], h_fp[:, hl], psS[:])
                spr = sp.tile([C, PN], dt.float32, tag="spr")
                nc.vector.tensor_copy(spr[:], psS[:])

                psR = psp.tile([C, PN], dt.float32, tag=pst)
                mm2(psR, WR, xt[:, hl], True, False)
                mm2(psR, UR, hsb[:, hl], False, True)
                rb = gp.tile([C, PN], dt.float32, tag="r")
                nc.scalar.activation(rb[:], psR[:], AF.Sigmoid,
                                     bias=bias[:, 3:4])

                psHH = psp.tile([C, PN], dt.float32, tag=pst)
                mm2(psHH, UH, hsb[:, hl], True, True)
                ru = gp.tile([C, PN], dt.bfloat16, tag="ru")
                nc.vector.tensor_mul(ru[:], rb[:], psHH[:])

                nc.gpsimd.tensor_add(hsde[:, hl], h_fp[:, hl], spr[:])

                psZ = psp.tile([C, PN], dt.float32, tag=pst)
                mm2(psZ, WZ, xt[:, hl], True, False)
                mm2(psZ, UZ, hsb[:, hl], False, True)
                zb = gp.tile([C, PN], dt.float32, tag="z")
                nc.scalar.activation(zb[:], psZ[:], AF.Sigmoid,
                                     bias=bias[:, 2:3])
                zc = gp.tile([C, PN], dt.float32, tag="zc")
                nc.vector.tensor_scalar(zc[:], zb[:], -1.0, 1.0,
                                        mybir.AluOpType.mult,
                                        mybir.AluOpType.add)
                uz = sp.tile([C, PN], dt.float32, tag="uz")
                nc.gpsimd.tensor_mul(uz[:], zb[:], hsde[:, hl])

                psHX = psp.tile([C, PN], dt.float32, tag=pst)
                mm2(psHX, WH, xt[:, hl], True, False)
                mm2(psHX, I_ID, ru[:], False, True)
                nc.scalar.activation(nf[:, hl], psHX[:], AF.Tanh,
                                     bias=bias[:, 4:5])

                vc = sp.tile([C, PN], dt.float32, tag="vc")
                nc.vector.tensor_mul(vc[:], zc[:], nf[:, hl])
                nc.vector.tensor_add(hnb[:, hl], vc[:], uz[:])
                nc.gpsimd.tensor_add(hnew[:, hl], vc[:], uz[:])

            nc.sync.dma_start(
                dz_d[t].rearrange("b c p -> c b p"),
                dzb[:].rearrange("c (b p) -> c b p", b=BL))
            nc.sync.dma_start(
                drift_d[t].rearrange("b c p -> c b p"),
                hnew[:].rearrange("c (b p) -> c b p", b=BL))
            h_fp = hnew
            h_bf = hnb

    nc.compile()
    return nc


def _get_nc():
    if "nc" not in _CACHE:
        _CACHE["nc"] = _build_nc()
    return _CACHE["nc"]


def _host_prep(track_loc, aa_out, noise, hidden, Wf, bf, Wg, bg,
               Wz, Uz, bz, Wr, Ur, br, Wh, Uh, bh):
    dts = _dts()
    sq = np.sqrt(np.abs(dts)).astype(np.float32)

    eye = np.eye(C, dtype=np.float32)
    wt = np.stack([
        np.asarray(Wf).T, np.asarray(Wg).T, np.asarray(Wz).T,
        np.asarray(Uz).T, np.asarray(Wr).T, np.asarray(Ur).T,
        np.asarray(Wh).T, np.asarray(Uh).T,
        eye * dts[0], eye * dts[1], eye,
    ]).astype(bf16)
    bias = np.stack([np.asarray(b, np.float32) for b in (bf, bg, bz, br, bh)],
                    axis=1)
    h0f = np.ascontiguousarray(
        np.broadcast_to(np.asarray(hidden, np.float32)[:, None], (C, PN)))
    h0b = h0f.astype(bf16)

    aa_bf = np.asarray(aa_out, np.float32).astype(bf16)
    nts_bf = (np.asarray(noise, np.float32)
              * sq[:, None, None, None]).astype(bf16)

    in_maps = []
    for k in range(N_CORES):
        sl = slice(BL * k, BL * (k + 1))
        in_maps.append({
            "aa": np.ascontiguousarray(aa_bf[:, sl]),
            "nts": np.ascontiguousarray(nts_bf[:, sl]),
            "h0f": h0f, "h0b": h0b, "wt": wt, "bias": bias,
        })
    return in_maps


def _assemble(results):
    hs = np.concatenate([r["drift"] for r in results], axis=1)   # (S,B,C,PN) f32
    dzs = np.concatenate([r["dz"].astype(np.float32) for r in results], axis=1)
    all_drift = np.ascontiguousarray(hs.transpose(1, 0, 2, 3))
    all_dz = np.ascontiguousarray(dzs.transpose(1, 0, 2, 3))
    stride = (STEPS - 1) // (TP - 1)
    drift_loc = np.ascontiguousarray(all_drift[:, ::stride])
    return drift_loc, all_drift, all_dz


def kernel(**inputs):
    from concourse import bass_utils
    nc = _get_nc()
    in_maps = _host_prep(**inputs)
    res = bass_utils.run_bass_kernel_spmd(nc, in_maps,
                                          core_ids=list(range(N_CORES)))
    return _assemble(res.results)


# revision 13
# speedup vs baseline: 1.7688x; 1.0536x over previous
"""Trainium2 Bass kernel for nn_LocalEncoderSDESepPara2 (SDE+GRU scan).

Full-input contract: kernel(**inputs) takes the unsharded numpy inputs and
returns (drift_loc, all_drift_loc, all_diff_z) exactly like the reference.

Sharding: data-parallel over B (16 batches -> 2 per core on 8 cores).  The
(C,C) weights are replicated; the 21-step scan runs locally per core.

Device math (per core, per step, state h: [C=128 partitions, 2048 cols]):
  drift  = tanh(Wf@h + bf)                (ACT, PSUM->SBUF bf16)
  diff   = sigmoid(Wg@h + bg)             (ACT, bf16)
  dz     = diff * (noise*sqrt|dt|)        (DVE bf16 2x; also an output)
  S      = (dt*I)@drift + I@dz            (PE identity matmuls, accumulated)
  h_sde  = h + S                          (DVE fp32)
  z      = sigmoid(Wz@x + Uz@h_sde + bz)  (PE accum + ACT, fp32)
  r      = sigmoid(Wr@x + Ur@h_sde + br)  (fp32)
  rU     = r * (Uh@h_sde)                 (DVE, out bf16)
  n      = tanh(Wh@x + I@rU + bh)         (PE accum + ACT, fp32)
  d      = h_sde - n                      (GPSIMD fp32)
  h_new  = n + z*d                        (DVE fp32)
Matmuls run in bf16 (inputs rounded), recurrent state stays fp32.
"""

import numpy as np
import ml_dtypes
from contextlib import ExitStack

B, C, TP, PN, STEPS = 16, 128, 11, 1024, 21
N_CORES = 8
BL = B // N_CORES          # batches per core = halves per core
F = BL * PN                # 2048 free columns per core
f16 = np.float16

# stationary-weight slots in the packed weight tensor
WF, WG, WZ, UZ, WR, UR, WH, UH, I_DT0, I_DTN, I_ID = range(11)
NW = 11

_CACHE = {}


def _dts():
    ts = np.linspace(TP - 1.0, 0.0, STEPS).astype(np.float32)
    return np.concatenate([np.full((1,), 0.01, np.float32), np.diff(ts)])


def _build_nc():
    import concourse.bacc as bacc
    import concourse.tile as tile
    from concourse import mybir

    dt = mybir.dt
    AF = mybir.ActivationFunctionType

    nc = bacc.Bacc("TRN2", target_bir_lowering=False, debug=False,
                   num_devices=N_CORES)
    aa = nc.dram_tensor("aa", (STEPS, BL, C, PN), dt.float16,
                        kind="ExternalInput").ap()
    nts = nc.dram_tensor("nts", (STEPS, BL, C, PN), dt.float16,
                         kind="ExternalInput").ap()
    h0f_d = nc.dram_tensor("h0f", (C, PN), dt.float32,
                           kind="ExternalInput").ap()
    h0b_d = nc.dram_tensor("h0b", (C, PN), dt.float16,
                           kind="ExternalInput").ap()
    wt_d = nc.dram_tensor("wt", (NW, C, C), dt.float16,
                          kind="ExternalInput").ap()
    bias_d = nc.dram_tensor("bias", (C, 5), dt.float32,
                            kind="ExternalInput").ap()
    drift_d = nc.dram_tensor("drift", (STEPS, BL, C, PN), dt.float32,
                             kind="ExternalOutput").ap()
    dz_d = nc.dram_tensor("dz", (STEPS, BL, C, PN), dt.float16,
                          kind="ExternalOutput").ap()

    with tile.TileContext(nc) as tc, ExitStack() as ctx:
        const = ctx.enter_context(tc.tile_pool(name="const", bufs=1))
        wt = const.tile([C, NW * C], dt.float16)
        nc.sync.dma_start(
            wt[:].rearrange("c (i o) -> c i o", i=NW),
            wt_d.rearrange("i c o -> c i o"),
        )
        bias = const.tile([C, 5], dt.float32)
        nc.sync.dma_start(bias[:], bias_d[:])
        h0f = const.tile([C, F], dt.float32)
        for hh in range(BL):
            nc.sync.dma_start(h0f[:, hh * PN:(hh + 1) * PN], h0f_d[:])
        h0b = const.tile([C, F], dt.float16)
        for hh in range(BL):
            nc.sync.dma_start(h0b[:, hh * PN:(hh + 1) * PN], h0b_d[:])

        iop = ctx.enter_context(tc.tile_pool(name="io", bufs=2))
        gp = ctx.enter_context(tc.tile_pool(name="gates", bufs=2))
        sp = ctx.enter_context(tc.tile_pool(name="state", bufs=2))
        hp = ctx.enter_context(tc.tile_pool(name="hstate", bufs=2))
        psA = ctx.enter_context(tc.tile_pool(name="psumA", bufs=2, space="PSUM"))
        psB = ctx.enter_context(tc.tile_pool(name="psumB", bufs=2, space="PSUM"))

        def W(i):
            return wt[:, i * C:(i + 1) * C]

        def mm2(ps, iw, rhs, start, stop):
            # ps: (C, PN) psum tile; rhs: (C, F) tile sliced per half `rhs_off`
            for s in range(2):
                nc.tensor.matmul(ps[:, s * 512:(s + 1) * 512], W(iw),
                                 rhs[:, s * 512:(s + 1) * 512],
                                 start=start, stop=stop)

        h_fp = h0f
        h_bf = h0b

        for t in range(STEPS):
            idt = I_DT0 if t == 0 else I_DTN
            xt = iop.tile([C, F], dt.float16, tag="xt")
            nc.sync.dma_start(
                xt[:].rearrange("c (b p) -> c b p", b=BL),
                aa[t].rearrange("b c p -> c b p"))
            nt = iop.tile([C, F], dt.float16, tag="nt")
            nc.sync.dma_start(
                nt[:].rearrange("c (b p) -> c b p", b=BL),
                nts[t].rearrange("b c p -> c b p"))

            dzb = gp.tile([C, F], dt.float16, tag="dz")
            hsb = gp.tile([C, F], dt.float16, tag="hsb")
            nf = sp.tile([C, F], dt.float16, tag="n")
            hnew = hp.tile([C, F], dt.float32, tag="hnew")
            hnb = hp.tile([C, F], dt.float16, tag="hnb")

            for hh in range(BL):
                hl = slice(hh * PN, (hh + 1) * PN)
                psp = psA if hh == 0 else psB
                pst = "psA" if hh == 0 else "psB"

                psG = psp.tile([C, PN], dt.float32, tag=pst)
                mm2(psG, WG, h_bf[:, hl], True, True)
                psF = psp.tile([C, PN], dt.float32, tag=pst)
                mm2(psF, WF, h_bf[:, hl], True, True)
                diff = gp.tile([C, PN], dt.float16, tag="diff")
                nc.scalar.activation(diff[:], psG[:], AF.Sigmoid,
                                     bias=bias[:, 1:2])
                drift = gp.tile([C, PN], dt.float16, tag="drift")
                nc.scalar.activation(drift[:], psF[:], AF.Tanh,
                                     bias=bias[:, 0:1])
                nc.vector.tensor_mul(dzb[:, hl], diff[:], nt[:, hl])

                psS = psp.tile([C, PN], dt.float32, tag=pst)
                mm2(psS, idt, drift[:], True, False)
                mm2(psS, I_ID, dzb[:, hl], False, True)
                nc.vector.tensor_add(hsb[:, hl], h_fp[:, hl], psS[:])

                psR = psp.tile([C, PN], dt.float32, tag=pst)
                mm2(psR, WR, xt[:, hl], True, False)
                mm2(psR, UR, hsb[:, hl], False, True)
                rb = gp.tile([C, PN], dt.float16, tag="r")
                nc.scalar.activation(rb[:], psR[:], AF.Sigmoid,
                                     bias=bias[:, 3:4])

                psHH = psp.tile([C, PN], dt.float32, tag=pst)
                mm2(psHH, UH, hsb[:, hl], True, True)
                ru = gp.tile([C, PN], dt.float16, tag="ru")
                nc.vector.tensor_mul(ru[:], rb[:], psHH[:])

                psZ = psp.tile([C, PN], dt.float32, tag=pst)
                mm2(psZ, WZ, xt[:, hl], True, False)
                mm2(psZ, UZ, hsb[:, hl], False, True)
                zb = gp.tile([C, PN], dt.float16, tag="z")
                nc.scalar.activation(zb[:], psZ[:], AF.Sigmoid,
                                     bias=bias[:, 2:3])
                zc = gp.tile([C, PN], dt.float16, tag="zc")
                nc.vector.tensor_scalar(zc[:], zb[:], -1.0, 1.0,
                                        mybir.AluOpType.mult,
                                        mybir.AluOpType.add)
                uz = gp.tile([C, PN], dt.float16, tag="uz")
                nc.vector.tensor_mul(uz[:], zb[:], hsb[:, hl])

                psHX = psp.tile([C, PN], dt.float32, tag=pst)
                mm2(psHX, WH, xt[:, hl], True, False)
                mm2(psHX, I_ID, ru[:], False, True)
                nc.scalar.activation(nf[:, hl], psHX[:], AF.Tanh,
                                     bias=bias[:, 4:5])

                vc = gp.tile([C, PN], dt.float16, tag="vc")
                nc.vector.tensor_mul(vc[:], zc[:], nf[:, hl])
                nc.vector.tensor_add(hnb[:, hl], vc[:], uz[:])
                nc.gpsimd.tensor_add(hnew[:, hl], vc[:], uz[:])

            nc.sync.dma_start(
                dz_d[t].rearrange("b c p -> c b p"),
                dzb[:].rearrange("c (b p) -> c b p", b=BL))
            nc.sync.dma_start(
                drift_d[t].rearrange("b c p -> c b p"),
                hnew[:].rearrange("c (b p) -> c b p", b=BL))
            h_fp = hnew
            h_bf = hnb

    nc.compile()
    return nc


def _get_nc():
    if "nc" not in _CACHE:
        _CACHE["nc"] = _build_nc()
    return _CACHE["nc"]


def _host_prep(track_loc, aa_out, noise, hidden, Wf, bf, Wg, bg,
               Wz, Uz, bz, Wr, Ur, br, Wh, Uh, bh):
    dts = _dts()
    sq = np.sqrt(np.abs(dts)).astype(np.float32)

    eye = np.eye(C, dtype=np.float32)
    wt = np.stack([
        np.asarray(Wf).T, np.asarray(Wg).T, np.asarray(Wz).T,
        np.asarray(Uz).T, np.asarray(Wr).T, np.asarray(Ur).T,
        np.asarray(Wh).T, np.asarray(Uh).T,
        eye * dts[0], eye * dts[1], eye,
    ]).astype(f16)
    bias = np.stack([np.asarray(b, np.float32) for b in (bf, bg, bz, br, bh)],
                    axis=1)
    h0f = np.ascontiguousarray(
        np.broadcast_to(np.asarray(hidden, np.float32)[:, None], (C, PN)))
    h0b = h0f.astype(f16)

    aa_bf = np.asarray(aa_out, np.float32).astype(f16)
    nts_bf = (np.asarray(noise, np.float32)
              * sq[:, None, None, None]).astype(f16)

    in_maps = []
    for k in range(N_CORES):
        sl = slice(BL * k, BL * (k + 1))
        in_maps.append({
            "aa": np.ascontiguousarray(aa_bf[:, sl]),
            "nts": np.ascontiguousarray(nts_bf[:, sl]),
            "h0f": h0f, "h0b": h0b, "wt": wt, "bias": bias,
        })
    return in_maps


def _assemble(results):
    hs = np.concatenate([r["drift"] for r in results], axis=1)   # (S,B,C,PN) f32
    dzs = np.concatenate([r["dz"].astype(np.float32) for r in results], axis=1)
    all_drift = np.ascontiguousarray(hs.transpose(1, 0, 2, 3))
    all_dz = np.ascontiguousarray(dzs.transpose(1, 0, 2, 3))
    stride = (STEPS - 1) // (TP - 1)
    drift_loc = np.ascontiguousarray(all_drift[:, ::stride])
    return drift_loc, all_drift, all_dz


def kernel(**inputs):
    from concourse import bass_utils
    nc = _get_nc()
    in_maps = _host_prep(**inputs)
    res = bass_utils.run_bass_kernel_spmd(nc, in_maps,
                                          core_ids=list(range(N_CORES)))
    return _assemble(res.results)
